# revision 44
# baseline (speedup 1.0000x reference)
"""Trainium2 Bass kernel for the CSMAdapter module.

Contract: kernel(**inputs) takes the FULL unsharded inputs (as produced by
the reference setup_inputs()) and returns the FULL output [4, 100, 1024].

Strategy
--------
All weight-only computation is folded on the host (it is data-independent):
    w_proj   = W_in @ Wd.T + bd
    w_prime  = P.T @ w_proj @ P
    masked_w = w_prime * sigmoid(spectral_mask)
    A        = P @ masked_w.T @ P.T          # fused = x @ A
    W_big    = W_in.T @ A                    # fused = llama @ W_big + b_in @ A
The final LayerNorm + mel projection algebra is folded into the mel GEMM:
    mel[m,t] = rstd[t]*(Wg @ h2)[m,t] - (mu[t]*rstd[t])*c1[m] + c2[m]
with Wg = Wmel * ln_g, c1 = Wmel @ ln_g, c2 = Wmel @ ln_b + bmel.

Device (SPMD over 8 cores, data-parallel over the 4096 tokens, 512 each +
2-token conv halos).  All heavy matmuls run in bf16 (1 cycle/row on the PE;
fp8 DoubleRow with error compensation was tried and loses: the ~150 ns
per-matmul instruction overhead floor dominates, and compensation needs
1.5x the instruction count).  PSUM accumulation stays fp32.

DMA: two ~95-115 GB/s HW DGE queues (sync + scalar engines) stream x first,
then wbig chunks, each tile split by partition halves across both queues;
the gpsimd software-DGE queue is a third lane carrying the consts, the last
wbig chunk and the conv/mel weights.  GEMM d-tiles are emitted in chunk-
arrival order (d0/d1 k-chunk-interleaved against the x stream, then d6/d7
from the gpsimd chunk, then d2/d3, d4/d5), with conv1 -> HW-Gelu -> conv2
-> stats software-pipelined behind.
"""

import sys

import numpy as np


def _ensure_concourse():
    try:
        import concourse  # noqa: F401
    except ImportError:  # pragma: no cover
        for p in ("/opt/trn_rl_repo", "/root/.axon_site/_ro/trn_rl_repo"):
            if p not in sys.path:
                sys.path.insert(0, p)


# ---- static shapes ----
B, T, L, D = 4, 1024, 3072, 1024
NCORES = 8
TOK = 512            # owned tokens per core
EXT = TOK + 4        # fused ext window: tokens -2 .. TOK+2
G1E = TOK + 2        # conv1 ext output: tokens -1 .. TOK+1
KT = L // 128        # 24
KH = KT // 2         # 12
DT = D // 128        # 8
NMEL = 100
GS = 64              # group size (1024 / 16 groups)
GROUPS_ = 16

# cb (per-partition constants) column layout
CB_B1 = 0            # conv1 bias (gelu bias), 8 cols
CB_B2S = 8           # conv2 bias / 32 (Square path), 8 cols
CB_B2 = 16           # conv2 bias, 8 cols
CB_EPS = 24          # LN eps
CB_BBIG = 25         # fused GEMM bias b_big, 8 cols
CB_LEN = 33

# smalls (f32r row vector) layout
OFF_C1 = 0
OFF_C2 = OFF_C1 + NMEL
OFF_ONES = OFF_C2 + NMEL
SM_LEN = OFF_ONES + TOK

LN_EPS = 1e-5
N_WU = 28            # PE warmup matmuls (pstate ramp during input DMA)

_PROGRAM = None          # cached program
LAST_RESULTS = None      # BassKernelResults of the most recent run (for test.py)


def _build_program():
    _ensure_concourse()
    from concourse import bacc, tile
    import concourse.mybir as mybir

    f32 = mybir.dt.float32
    f32r = mybir.dt.float32r
    bf16 = mybir.dt.bfloat16
    AF = mybir.ActivationFunctionType
    MUL = mybir.AluOpType.mult

    nc = bacc.Bacc("TRN2", debug=False, target_bir_lowering=False)

    # DRAM layouts are partition-major so every DMA is contiguous.
    xt_d = nc.dram_tensor("xt", [2, 128, KH, EXT], bf16, kind="ExternalInput")
    wbig_d = nc.dram_tensor("wbig", [4, 128, 2 * KT, 128], bf16,
                            kind="ExternalInput")
    cw1_d = nc.dram_tensor("cw1", [128, DT, 3, 128], bf16, kind="ExternalInput")
    cw2_d = nc.dram_tensor("cw2", [128, DT, 3, 128], bf16, kind="ExternalInput")
    wgt_d = nc.dram_tensor("wgt", [128, DT, NMEL], bf16, kind="ExternalInput")
    cb_d = nc.dram_tensor("cb", [128, CB_LEN], f32, kind="ExternalInput")
    sm_d = nc.dram_tensor("smalls", [1, SM_LEN], f32r, kind="ExternalInput")
    smb_d = nc.dram_tensor("smb", [1, 2 * NMEL], bf16, kind="ExternalInput")
    onec_d = nc.dram_tensor("onec", [128, 1], bf16, kind="ExternalInput")
    # host-computed halo columns: per d-tile, 4 fused halo cols + 2 g halo cols
    halo_d = nc.dram_tensor("halo", [128, DT, 6], bf16, kind="ExternalInput")
    mel_d = nc.dram_tensor("mel", [NMEL, TOK], f32, kind="ExternalOutput")

    with tile.TileContext(nc) as tc:
        with (
            tc.tile_pool(name="consts", bufs=1) as consts,
            tc.tile_pool(name="wpool", bufs=1) as wpool,
            tc.tile_pool(name="acts", bufs=1) as acts,
            tc.tile_pool(name="stats", bufs=1) as stats,
            tc.tile_pool(name="ps_mm", bufs=5, space="PSUM") as ps_mm,
            tc.tile_pool(name="ps_sq", bufs=1, space="PSUM") as ps_sqp,
            tc.tile_pool(name="ps_mel", bufs=1, space="PSUM") as ps_melp,
            tc.tile_pool(name="ps_r", bufs=1, space="PSUM") as ps_rp,
        ):
            # ---- warmup tile via memset (no DMA dependency) ----
            wu_sb = consts.tile([128, TOK], bf16, name="wu_sb")
            nc.vector.memset(wu_sb, 0.0)
            sm_sb = consts.tile([1, SM_LEN], f32r, name="sm_sb")
            nc.gpsimd.dma_start(out=sm_sb, in_=sm_d[:])
            cb_sb = consts.tile([128, CB_LEN], f32, name="cb_sb")
            nc.gpsimd.dma_start(out=cb_sb, in_=cb_d[:])
            ones_col = consts.tile([128, 1], bf16, name="ones_col")
            nc.gpsimd.dma_start(out=ones_col, in_=onec_d[:])
            halo_sb = consts.tile([128, DT, 6], bf16, name="halo_sb")
            nc.gpsimd.dma_start(out=halo_sb, in_=halo_d[:])
            smb_sb = consts.tile([1, 2 * NMEL], bf16, name="smb_sb")
            nc.gpsimd.dma_start(out=smb_sb, in_=smb_d[:])
            ones_row = sm_sb[0:1, OFF_ONES : OFF_ONES + TOK]

            # ---- PE warmup (pstate ramp) while input DMAs stream ----
            ps_wu = ps_mm.tile([128, TOK], f32, name="ps_wu", tag="mm")
            for i in range(N_WU):
                nc.tensor.matmul(
                    ps_wu[:, 0:256], lhsT=wu_sb[:, 0:128],
                    rhs=wu_sb[:, 0:256],
                    start=(i == 0), stop=(i == N_WU - 1),
                )

            # rank-1 constant part of the output correction, started early:
            # ps_r = c2 (x) ones  (+ c1 (x) negu at the tail)
            ps_r = ps_rp.tile([NMEL, TOK], f32, name="ps_r")
            nc.tensor.matmul(
                ps_r, lhsT=sm_sb[0:1, OFF_C2 : OFF_C2 + NMEL],
                rhs=ones_row, start=True, stop=False,
            )

            # ---- input DMAs (see module docstring) ----
            def load_split(t, src_ap):
                nc.sync.dma_start(out=t[0:64], in_=src_ap[0:64])
                nc.scalar.dma_start(out=t[64:128], in_=src_ap[64:128])

            wbc = [
                wpool.tile([128, 2 * KT, 128], bf16, name=f"wbc{c}",
                           tag=f"wbc{c}")
                for c in range(4)
            ]
            xg = [
                consts.tile([128, KH, EXT], bf16, name=f"xg{j}", tag=f"xg{j}")
                for j in range(2)
            ]
            # HW queues: wbc0 first, then all x in 6-k sub-chunks, then the
            # remaining HW-side wbig chunks.
            load_split(wbc[0], wbig_d[0])
            for j in range(2):
                for a in range(0, KH, 4):
                    load_split(xg[j][:, a : a + 4, :],
                               xt_d[j][:, a : a + 4, :])
            load_split(wbc[1], wbig_d[1])
            load_split(wbc[2], wbig_d[2])
            # gpsimd lane: wbc3 early (d6/d7), then conv/mel weights
            cw1_sb = consts.tile([128, DT, 3, 128], bf16, name="cw1_sb")
            cw2_sb = consts.tile([128, DT, 3, 128], bf16, name="cw2_sb")
            wgt_sb = consts.tile([128, DT, NMEL], bf16, name="wgt_sb")
            nc.gpsimd.dma_start(out=wbc[3], in_=wbig_d[3])
            nc.gpsimd.dma_start(out=cw1_sb, in_=cw1_d[:])
            nc.gpsimd.dma_start(out=cw2_sb, in_=cw2_d[:])
            nc.gpsimd.dma_start(out=wgt_sb, in_=wgt_d[:])

            # preload the scalar-engine Sqrt table during the DMA-bound
            # front.  The act-table SRAM holds ~2 tables; with the plain
            # bias-adds moved to the vector engine the scalar working set is
            # exactly {Gelu+Square, Sqrt}, so nothing evicts it and no
            # ~1.3us ACT_TABLE_LOAD lands on the LN critical path.
            pre = stats.tile([1, 8], f32, name="pre", tag="pre", bufs=2)
            nc.scalar.activation(pre, wu_sb[0:1, 0:8], AF.Sqrt)

            def xk(k):
                return xg[k // KH][:, k % KH, :]

            fused = [None] * DT
            g = [None] * DT
            h2 = [None] * DT
            h2sq = [None] * DT
            ps_sq_ref = [None]
            ps_m_ref = [None]
            psA = {}

            def gemm_chunk(d, ka, kb):
                if d not in psA:
                    psA[d] = ps_mm.tile([128, TOK], f32, name=f"psA{d}",
                                        tag="mm")
                for k in range(ka, kb):
                    nc.tensor.matmul(
                        psA[d],
                        lhsT=wbc[d // 2][:, (d % 2) * KT + k, :],
                        rhs=xk(k)[:, 2 : 2 + TOK],
                        start=(k == 0), stop=(k == KT - 1),
                    )

            def fu_copy(d):
                fu = acts.tile([128, EXT], bf16, name=f"fu{d}", tag=f"fu{d}")
                fused[d] = fu
                nc.scalar.add(out=fu[:, 2 : 2 + TOK], in_=psA[d],
                              add=cb_sb[:, CB_BBIG + d : CB_BBIG + d + 1])
                nc.vector.tensor_copy(fu[:, 0:2], halo_sb[:, d, 0:2])
                nc.vector.tensor_copy(fu[:, EXT - 2 : EXT], halo_sb[:, d, 2:4])

            def conv1(d):
                # device computes g_ext cols [1, 513); cols 0 and 513 from host
                gd = acts.tile([128, G1E], bf16, name=f"g{d}", tag=f"g{d}")
                g[d] = gd
                ps = ps_mm.tile([128, TOK], f32, name=f"psB{d}", tag="mm")
                for tap in range(3):
                    nc.tensor.matmul(
                        ps, lhsT=cw1_sb[:, d, tap, :],
                        rhs=fused[d][:, 1 + tap : 1 + tap + TOK],
                        start=(tap == 0), stop=(tap == 2),
                    )
                nc.scalar.activation(
                    out=gd[:, 1 : 1 + TOK], in_=ps, func=AF.Gelu,
                    bias=cb_sb[:, CB_B1 + d : CB_B1 + d + 1], scale=1.0,
                )
                nc.vector.tensor_copy(gd[:, 0:1], halo_sb[:, d, 4:5])
                nc.vector.tensor_copy(gd[:, G1E - 1 : G1E], halo_sb[:, d, 5:6])

            def conv2(d):
                h2d = acts.tile([128, TOK], bf16, name=f"h2{d}", tag=f"h2{d}")
                h2sqd = acts.tile([128, TOK], bf16, name=f"h2sq{d}", tag="h2sq",
                                  bufs=2)
                h2[d] = h2d
                h2sq[d] = h2sqd
                ps = ps_mm.tile([128, TOK], f32, name=f"psC{d}", tag="mm")
                for tap in range(3):
                    nc.tensor.matmul(
                        ps, lhsT=cw2_sb[:, d, tap, :],
                        rhs=g[d][:, tap : tap + TOK],
                        start=(tap == 0), stop=(tap == 2),
                    )
                nc.vector.tensor_scalar_add(
                    h2d, ps, cb_sb[:, CB_B2 + d : CB_B2 + d + 1])
                # h2sq = ((ps + b2)/32)^2 = h2^2 / 1024 -> ps_sq row = E[h2^2]
                nc.scalar.activation(
                    out=h2sqd, in_=ps, func=AF.Square,
                    bias=cb_sb[:, CB_B2S + d : CB_B2S + d + 1], scale=0.03125,
                )

            stat_n = [0]

            def statmm(d):
                first = stat_n[0] == 0
                stat_n[0] += 1
                last = stat_n[0] == DT
                if first:
                    ps_sq_ref[0] = ps_sqp.tile([33, TOK], f32, name="ps_sq")
                    ps_m_ref[0] = ps_melp.tile([NMEL, TOK], f32, name="ps_m")
                nc.tensor.matmul(ps_sq_ref[0][0:1, :], lhsT=ones_col,
                                 rhs=h2sq[d][:], start=first, stop=last)
                nc.tensor.matmul(ps_sq_ref[0][32:33, :], lhsT=ones_col,
                                 rhs=h2[d][:], start=first, stop=last)
                nc.tensor.matmul(ps_m_ref[0], lhsT=wgt_sb[:, d, :], rhs=h2[d][:],
                                 start=first, stop=last)

            # ---- emission in expected DMA-arrival order ----
            # d0/d1 interleave against the early x sub-chunks; d6/d7 (whose
            # weights arrive early on the gpsimd lane) join to fill the gaps
            # between x sub-chunk arrivals; d2..d5 follow weight arrival.
            for c in range(3):
                gemm_chunk(0, 4 * c, 4 * c + 4)
                gemm_chunk(1, 4 * c, 4 * c + 4)
            for c in range(3):
                gemm_chunk(6, 4 * c, 4 * c + 4)
                gemm_chunk(7, 4 * c, 4 * c + 4)
            for c in (3, 4, 5):
                for d in (0, 1, 6, 7):
                    gemm_chunk(d, 4 * c, 4 * c + 4)
            fu_copy(0)
            fu_copy(1)
            fu_copy(6)
            fu_copy(7)
            conv1(0)
            gemm_chunk(2, 0, KT)
            fu_copy(2)
            conv1(1)
            gemm_chunk(3, 0, KT)
            fu_copy(3)
            conv1(6)
            conv2(0)
            gemm_chunk(4, 0, KT)
            fu_copy(4)
            conv1(7)
            conv2(1)
            statmm(0)
            gemm_chunk(5, 0, KT)
            fu_copy(5)
            conv1(2)
            conv2(6)
            statmm(1)
            conv1(3)
            conv2(7)
            statmm(6)
            conv1(4)
            conv2(2)
            statmm(7)
            conv1(5)
            conv2(3)
            statmm(2)
            conv2(4)
            statmm(3)
            conv2(5)
            statmm(4)
            statmm(5)

            # ---- LN stats on [1, TOK] ----
            ps_sq = ps_sq_ref[0][0:1, :]     # E[h2^2] per token
            ps_m = ps_m_ref[0]
            mu_row = ps_sq_ref[0][32:33, :]  # sum h2; x(1/D) folded into ops
            msq = stats.tile([1, TOK], f32, name="msq")
            nc.scalar.activation(msq, mu_row, AF.Square, scale=1.0 / D)
            var = stats.tile([1, TOK], f32, name="var", tag="sv", bufs=2)
            nc.vector.scalar_tensor_tensor(
                var, in0=ps_sq, scalar=1.0, in1=msq,
                op0=MUL, op1=mybir.AluOpType.subtract,
            )
            sqv = stats.tile([1, TOK], f32, name="sqv", tag="sv", bufs=2)
            nc.scalar.activation(sqv, var, AF.Sqrt,
                                 bias=cb_sb[0:1, CB_EPS : CB_EPS + 1], scale=1.0)
            rstd32 = stats.tile([1, TOK], f32, name="rstd32", tag="sv", bufs=2)
            nc.vector.reciprocal_approx_fast(rstd32, sqv)
            rstd = stats.tile([1, TOK], bf16, name="rstd")
            nc.vector.tensor_copy(rstd, rstd32)
            negu = stats.tile([1, TOK], bf16, name="negu")
            nc.vector.scalar_tensor_tensor(
                negu, in0=mu_row, scalar=-1.0 / D, in1=rstd32, op0=MUL, op1=MUL,
            )

            # ---- rank-1 corrections + output ----
            nc.tensor.matmul(
                ps_r, lhsT=smb_sb[0:1, 0:NMEL],
                rhs=negu, start=False, stop=True,
            )
            ps_s = ps_mm.tile([NMEL, TOK], f32, name="ps_s", tag="mm")
            nc.tensor.matmul(
                ps_s, lhsT=smb_sb[0:1, NMEL : 2 * NMEL],
                rhs=rstd, start=True, stop=True,
            )
            s_sb = stats.tile([NMEL, TOK], f32, name="s_sb")
            nc.vector.tensor_copy(s_sb, ps_s)
            out_sb = stats.tile([NMEL, TOK], f32, name="out_sb")
            nc.vector.tensor_mul(out_sb, ps_m[0:NMEL, :], s_sb)
            nc.vector.tensor_add(out_sb[0:64], out_sb[0:64], ps_r[0:64])
            nc.sync.dma_start(out=mel_d[0:64], in_=out_sb[0:64])
            nc.vector.tensor_add(out_sb[64:NMEL], out_sb[64:NMEL],
                                 ps_r[64:NMEL])
            nc.scalar.dma_start(out=mel_d[64:NMEL], in_=out_sb[64:NMEL])

    nc.compile()
    return nc


def _sigmoid64(x):
    return 1.0 / (1.0 + np.exp(-x.astype(np.float64)))


def _bf16(a):
    import ml_dtypes

    return np.ascontiguousarray(np.asarray(a, dtype=np.float32)).astype(
        ml_dtypes.bfloat16
    )


def host_prep(inputs):
    """Fold all data-independent computation; build per-core device inputs.

    Returns (shared, per_core) where shared is a dict of replicated arrays
    and per_core is a list of 8 dicts with the core-specific arrays.
    """
    f32 = np.float32
    W_in = np.asarray(inputs["W_in"], dtype=np.float64)
    Wd = np.asarray(inputs["Wd"], dtype=np.float64)
    bd = np.asarray(inputs["bd"], dtype=np.float64)
    P = np.asarray(inputs["P"], dtype=np.float64)
    smask = np.asarray(inputs["spectral_mask"], dtype=np.float64)
    b_in = np.asarray(inputs["b_in"], dtype=np.float64)

    w_proj = W_in @ Wd.T + bd[None, :]
    w_prime = P.T @ w_proj @ P
    masked_w = w_prime * _sigmoid64(smask)
    A = P @ masked_w.T @ P.T
    W_big64 = W_in.T @ A                                       # [L, D] f64
    b_big64 = b_in @ A                                         # [D] f64
    W_big = np.ascontiguousarray(W_big64, dtype=f32)

    # [chunk of 2 d-tiles, kp, (d%2)*KT + ktile, dc] (partition-major)
    wbig_t = _bf16(
        W_big.reshape(KT, 128, 4, 2, 128).transpose(2, 1, 3, 0, 4)
    ).reshape(4, 128, 2 * KT, 128)

    def blockdiag(w):
        w = np.asarray(w, dtype=f32)  # [C, GS, 3]
        out = np.zeros((DT, 3, 128, 128), dtype=f32)
        for d in range(DT):
            for co in range(128):
                c = d * 128 + co
                blk = co // GS
                # out[d, tap, blk*GS + i, co] = w[c, i, tap]
                out[d, :, blk * GS : (blk + 1) * GS, co] = w[c].T
        return out

    cw1_t = _bf16(blockdiag(inputs["conv1_w"]).transpose(2, 0, 1, 3))
    cw2_t = _bf16(blockdiag(inputs["conv2_w"]).transpose(2, 0, 1, 3))

    Wmel = np.asarray(inputs["Wmel"], dtype=np.float64)
    ln_g = np.asarray(inputs["ln_g"], dtype=np.float64)
    ln_b = np.asarray(inputs["ln_b"], dtype=np.float64)
    bmel = np.asarray(inputs["bmel"], dtype=np.float64)
    Wg = (Wmel * ln_g[None, :]).astype(f32)                    # [NMEL, D]
    wgt_t = _bf16(Wg.T.reshape(DT, 128, NMEL).transpose(1, 0, 2))
    c1 = (Wmel @ ln_g).astype(f32)
    c2 = (Wmel @ ln_b + bmel).astype(f32)

    cb_base = np.zeros((128, CB_LEN), dtype=f32)
    b1_cols = np.asarray(inputs["conv1_b"], dtype=f32).reshape(DT, 128).T
    b2_cols = np.asarray(inputs["conv2_b"], dtype=f32).reshape(DT, 128).T
    cb_base[:, CB_B1 : CB_B1 + DT] = b1_cols
    cb_base[:, CB_B2S : CB_B2S + DT] = b2_cols * np.float32(0.03125)
    cb_base[:, CB_B2 : CB_B2 + DT] = b2_cols
    cb_base[:, CB_EPS] = LN_EPS
    cb_base[:, CB_BBIG : CB_BBIG + DT] = b_big64.astype(f32).reshape(DT, 128).T

    llama = np.asarray(inputs["llama_embeddings"], dtype=f32).reshape(B * T, L)
    conv1_w_np = np.asarray(inputs["conv1_w"], dtype=np.float64)  # [D, GS, 3]
    conv1_b_np = np.asarray(inputs["conv1_b"], dtype=np.float64)
    gidx = np.arange(D) // GS

    import math
    _erf_vec = np.vectorize(math.erf)

    def _gelu64(x):
        return x * 0.5 * (1.0 + _erf_vec(x / math.sqrt(2.0)))

    shared = dict(wbig=wbig_t, cw1=cw1_t, cw2=cw2_t, wgt=wgt_t,
                  onec=_bf16(np.ones((128, 1), dtype=f32)))
    per_core = []
    for c in range(NCORES):
        b, h = divmod(c, 2)
        start = b * T + h * TOK
        ext_idx = np.arange(start - 2, start + TOK + 2)
        valid = (ext_idx >= b * T) & (ext_idx < (b + 1) * T)
        xext = np.zeros((EXT, L), dtype=f32)
        xext[valid] = llama[ext_idx[valid]]
        xt = _bf16(
            xext.T.reshape(2, KH, 128, EXT).transpose(0, 2, 1, 3)
        )  # [j, p, kk, t]

        # host-computed halo columns (exact fp32-grade)
        def fcol(u):
            gu = start + u
            if b * T <= gu < (b + 1) * T:
                return llama[gu].astype(np.float64) @ W_big64 + b_big64
            return np.zeros(D, dtype=np.float64)

        def conv1col(m3):
            # m3: [D, 3] inputs for taps 0..2 -> conv1 + bias, gelu
            in_g = m3.reshape(GROUPS_, GS, 3)[gidx]       # [D, GS, 3]
            out = np.einsum("cit,cit->c", conv1_w_np, in_g) + conv1_b_np
            return _gelu64(out)

        fm2, fm1, f0 = fcol(-2), fcol(-1), fcol(0)
        f510, f511 = fcol(510), fcol(511)
        f512, f513 = fcol(TOK), fcol(TOK + 1)
        if h == 1:
            g_left = conv1col(np.stack([fm2, fm1, f0], axis=1))
        else:
            g_left = np.zeros(D, dtype=np.float64)
        if h == 0:
            g_right = conv1col(np.stack([f511, f512, f513], axis=1))
        else:
            g_right = np.zeros(D, dtype=np.float64)
        halo = np.zeros((128, DT, 6), dtype=f32)
        for dd in range(DT):
            slc = slice(dd * 128, (dd + 1) * 128)
            halo[:, dd, 0] = fm2[slc]
            halo[:, dd, 1] = fm1[slc]
            halo[:, dd, 2] = f512[slc]
            halo[:, dd, 3] = f513[slc]
            halo[:, dd, 4] = g_left[slc]
            halo[:, dd, 5] = g_right[slc]

        sm = np.zeros((1, SM_LEN), dtype=f32)
        sm[0, OFF_C1 : OFF_C1 + NMEL] = c1
        sm[0, OFF_C2 : OFF_C2 + NMEL] = c2
        sm[0, OFF_ONES : OFF_ONES + TOK] = 1.0
        smb = np.zeros((1, 2 * NMEL), dtype=f32)
        smb[0, 0:NMEL] = c1
        smb[0, NMEL : 2 * NMEL] = 1.0

        per_core.append(dict(xt=xt, smalls=sm, cb=cb_base, halo=_bf16(halo),
                             smb=_bf16(smb)))
    return shared, per_core


def _ensure_axon_hooks():
    """If this image's antenv lacks axon_hooks (needed by bass_utils when
    BASS_TRACE is set under axon), register a functional ctypes-based hook so
    tracing degrades gracefully instead of crashing."""
    try:
        import antenv.axon_hooks  # noqa: F401
        return
    except ImportError:
        pass
    try:
        import contextlib
        import ctypes
        import types

        hook = None
        try:
            lib = ctypes.CDLL("/opt/axon/libaxon_pjrt.so")
            if hasattr(lib, "axon_start_nrt_profile"):
                lib.axon_start_nrt_profile.argtypes = [
                    ctypes.POINTER(ctypes.c_int64),
                    ctypes.c_size_t,
                ]
                lib.axon_start_nrt_profile.restype = ctypes.c_int64
                lib.axon_stop_nrt_profile.argtypes = [ctypes.c_char_p]
                lib.axon_stop_nrt_profile.restype = ctypes.c_int64

                @contextlib.contextmanager
                def hook(output_dir, device_ids):
                    import jax

                    jax.devices()
                    if device_ids:
                        ids = (ctypes.c_int64 * len(device_ids))(*device_ids)
                        rc = lib.axon_start_nrt_profile(ids, len(device_ids))
                    else:
                        rc = lib.axon_start_nrt_profile(None, 0)
                    if rc != 0:
                        raise RuntimeError(f"axon_start_nrt_profile rc={rc}")
                    try:
                        yield
                    finally:
                        lib.axon_stop_nrt_profile(str(output_dir).encode())
        except OSError:
            hook = None

        mod = types.ModuleType("antenv.axon_hooks")
        mod.get_axon_ntff_profile_hook = lambda: hook
        mod.set_axon_ntff_profile_hook = lambda h: None
        sys.modules["antenv.axon_hooks"] = mod
        import antenv

        antenv.axon_hooks = mod
    except Exception:
        pass


def kernel(**inputs):
    global _PROGRAM, LAST_RESULTS
    _ensure_concourse()
    _ensure_axon_hooks()
    from concourse import bass_utils

    if _PROGRAM is None:
        _PROGRAM = _build_program()
    nc = _PROGRAM

    shared, per_core = host_prep(inputs)
    in_maps = [{**shared, **pc} for pc in per_core]

    res = None
    last_exc = None
    for _attempt in range(3):
        try:
            res = bass_utils.run_bass_kernel_spmd(
                nc, in_maps, core_ids=list(range(NCORES))
            )
            break
        except Exception as exc:  # transient NRT device errors happen
            last_exc = exc
    if res is None:
        raise last_exc
    LAST_RESULTS = res

    out = np.zeros((B, NMEL, T), dtype=np.float32)
    for c in range(NCORES):
        b, h = divmod(c, 2)
        out[b, :, h * TOK : (h + 1) * TOK] = res.results[c]["mel"]
    return out
